# revision 31
# baseline (speedup 1.0000x reference)
"""TRN2 Bass kernel for nn_MultiHeadAttention_86878598464357.

reference:  qkv = x @ w_qkv.T (RoPE on q,k) -> causal softmax attention ->
            torch-faithful reshape [B,H,T,D]->[B,T,C] -> proj @ w_proj.T

Sharding (8 NeuronCores): tensor-parallel over heads, 2 heads per core.
Because the torch-faithful reshape makes output row t' depend only on head
t'//128, each core independently computes full output rows for its heads --
no collectives. Per core:
  - qkv projection for its 2 heads (bf16 matmuls, fp32 PSUM accumulation)
  - RoPE via sign-folded sin/cos tables (4 vector ops per chunk)
  - causal attention in transposed-score layout S^T[s,t] (no max-subtraction;
    scores are O(6) so exp is safe in fp32); softmax denominator via
    vector-accumulated exp tiles + one gpsimd partition_all_reduce per block
    (keeps the tensor engine free of M=1 denominator matmuls)
  - output projection with stride-16 lhsT access patterns implementing the
    reshape; w_proj slices stay resident in SBUF across both batches
Startup: x chunk 0 is DMA'd first (DGE completes transfers in FIFO order),
weights arrive in per-slice DMAs so the first matmul starts ~15us in.
Host side: transpose/cast inputs to bf16, build rope tables, scatter/gather.
"""
import math
from contextlib import ExitStack

import numpy as np

B, T, C = 2, 2048, 2048
H, D = 16, 128
HL = 2
TB = 512
NTB = T // TB
NTT = T // 128
KT = C // 128
KTH = KT // 2
SCALE = 1.0 / math.sqrt(D)
N_CORES = 8

_CACHE = {}


def _interleave(gen_a, gen_b, ratio):
    a = list(gen_a)
    bs = list(gen_b)
    bi = 0
    for i, chunk in enumerate(a):
        chunk()
        take = int(round((i + 1) * ratio)) - int(round(i * ratio))
        for _ in range(take):
            if bi < len(bs):
                bs[bi]()
                bi += 1
    while bi < len(bs):
        bs[bi]()
        bi += 1


def _emit(nc, io, p, mybir):
    """Emit the full per-core forward pass."""
    F32 = mybir.dt.float32
    BF16 = mybir.dt.bfloat16
    from concourse.bass_isa import ReduceOp

    w_sb = p["const"].tile([128, KT, 6 * 128], BF16, name="w_sb")
    cos2 = p["const"].tile([128, T], F32, name="cos2")
    sin2 = p["const"].tile([128, T], F32, name="sin2")
    tri_sb = p["const"].tile([128, 128], BF16, name="tri_sb")
    ones_sb = p["const"].tile([128, 1], BF16, name="ones_sb")
    nc.vector.memset(ones_sb[:], 1.0)



    x_holder = {}

    def load_x(b, tb, first=False):
        def f():
            xa = p["xp"].tile([128, KTH, TB], BF16, name="x_sb_a")
            xb = p["xp"].tile([128, KTH, TB], BF16, name="x_sb_b")
            src = io["x_bf"][b, tb]
            if first:
                # stage the first paired chains' inputs in dependency order:
                # quarter x, q-weight slices, rest of x
                nc.sync.dma_start(out=xa[:, 0 : KTH // 2],
                                  in_=src[:, 0 : KTH // 2])
                nc.sync.dma_start(out=w_sb[:, :, 0:128],
                                  in_=io["w_qkv_bf"][0])
                nc.sync.dma_start(out=w_sb[:, :, 128:256],
                                  in_=io["w_qkv_bf"][1])
                nc.sync.dma_start(out=xa[:, KTH // 2 : KTH],
                                  in_=src[:, KTH // 2 : KTH])
            else:
                nc.sync.dma_start(out=xa[:], in_=src[:, 0:KTH])
            nc.sync.dma_start(out=xb[:], in_=src[:, KTH:KT])
            x_holder[(b, tb)] = (xa, xb)
        return f

    def xslice(b, tb, kt):
        xa, xb = x_holder[(b, tb)]
        return (xa if kt < KTH else xb)[:, kt % KTH]

    def load_consts():
        # per-slice weight loads so early qk chains wait on little data; the
        # rope tables ship their first TB columns early (tb=0 ropes gate the
        # ps_mm recycle for the 3rd qk chain) and the rest after the weights
        nc.sync.dma_start(out=cos2[:, 0:TB], in_=io["cos2"][:, 0:TB])
        nc.sync.dma_start(out=sin2[:, 0:TB], in_=io["sin2"][:, 0:TB])
        for i in range(2, 6):
            nc.sync.dma_start(
                out=w_sb[:, :, i * 128 : (i + 1) * 128],
                in_=io["w_qkv_bf"][i],
            )
        nc.sync.dma_start(out=cos2[:, TB:T], in_=io["cos2"][:, TB:T])
        nc.sync.dma_start(out=sin2[:, TB:T], in_=io["sin2"][:, TB:T])
        nc.sync.dma_start(out=tri_sb[:], in_=io["tri"][:])

    def wslice(kt, fb):
        return w_sb[:, kt, fb * 128 : (fb + 1) * 128]

    qkv_t = {}
    out_sb = {}

    def qkv_chunks(b, skip_first_load=False):
        q = {h: p["qkvp"].tile([128, T], BF16, name=f"q_sb_{h}") for h in range(HL)}
        k = {h: p["qkvp"].tile([128, T], BF16, name=f"k_sb_{h}") for h in range(HL)}
        v = p["qkvp"].tile([128, NTT, HL * 128], BF16, name="v_sb")
        qkv_t[b] = (q, k, v)

        def rope(psum, dst, h, ts):
            # rope: dst[0:64]  = p0*cos - p1*sin
            #       dst[64:]   = p1*cos + p0*sin
            # cos2 = [cos;cos], sin2 = [-sin;sin]
            ta = p["misc"].tile([128, TB], F32, name="rope_ta")
            tb2 = p["misc"].tile([128, TB], BF16, name="rope_tb")
            nc.vector.tensor_mul(ta[:], psum[:], cos2[:, ts])
            nc.vector.tensor_mul(tb2[0:64], psum[64:128], sin2[0:64, ts])
            nc.vector.tensor_mul(tb2[64:128], psum[0:64], sin2[64:128, ts])
            nc.vector.tensor_add(dst[h][:, ts], ta[:], tb2[:])

        def qk_chunk(tb, fb, dst, h):
            def f():
                ts = slice(tb * TB, (tb + 1) * TB)
                psum = p["ps_mm"].tile([128, TB], F32, name="qk_psum", tag="mmps")
                for kt in range(KT):
                    nc.tensor.matmul(
                        psum[:],
                        wslice(kt, fb),
                        rhs=xslice(b, tb, kt),
                        start=(kt == 0),
                        stop=(kt == KT - 1),
                    )
                rope(psum, dst, h, ts)
            return f

        def qk_chunk_pair(tb, fb0_, fb1_, dst):
            # kt-interleaved pair: while the x chunk is still streaming in,
            # each arriving kt slice feeds two matmuls, matching the PE's
            # consumption rate to DMA bandwidth
            def f():
                ts = slice(tb * TB, (tb + 1) * TB)
                ps0 = p["ps_mm"].tile([128, TB], F32, name="qk_psum", tag="mmps")
                ps1 = p["ps_mm"].tile([128, TB], F32, name="qk_psum", tag="mmps")
                for kt in range(KT):
                    for ps_, fb in ((ps0, fb0_), (ps1, fb1_)):
                        nc.tensor.matmul(
                            ps_[:],
                            wslice(kt, fb),
                            rhs=xslice(b, tb, kt),
                            start=(kt == 0),
                            stop=(kt == KT - 1),
                        )
                rope(ps0, dst, 0, ts)
                rope(ps1, dst, 1, ts)
            return f

        def v_chunk(tb, tl):
            def f():
                tt = tb * 4 + tl
                psum = p["ps_mm"].tile([128, HL * 128], F32, name="v_psum",
                                       tag="mmps")
                for kt in range(KT):
                    nc.tensor.matmul(
                        psum[:],
                        xslice(b, tb, kt)[:, tl * 128 : (tl + 1) * 128],
                        rhs=w_sb[:, kt, 4 * 128 : 6 * 128],
                        start=(kt == 0),
                        stop=(kt == KT - 1),
                    )
                nc.scalar.copy(v[:, tt], psum[:])
            return f

        for tb in range(NTB):
            if tb == 0 and skip_first_load:
                yield qk_chunk_pair(0, 0, 1, q)
                yield qk_chunk(0, 2, k, 0)
                yield qk_chunk(0, 3, k, 1)
            else:
                yield load_x(b, tb)
                for fb, (dst, h) in enumerate([(q, 0), (q, 1), (k, 0), (k, 1)]):
                    yield qk_chunk(tb, fb, dst, h)
            for tl in range(4):
                yield v_chunk(tb, tl)

    def attn_blocks(b):
        # the denominator chain of block n is software-pipelined across the
        # next two blocks so neither the in-order PE queue nor the vector
        # queue ever waits on it:
        #   block n+1 start: den matmul (PE, eacc ready by then) + copy
        #   block n+1 end:   reciprocal (vector) + partition broadcast
        #   block n+2 start: o_ps normalize multiply (vector)
        pend_den, pend1, pend2 = [], [], []

        def flush():
            while pend_den:
                pend_den.pop(0)()
            while pend1:
                pend1.pop(0)()
            while pend2:
                pend2.pop(0)()

        for h in range(HL):
            o_sb = p["outp"].tile([128, T], BF16, name=f"o_sb_{b}_{h}")
            out_sb[(b, h)] = o_sb
            for tb in range(NTB):
                def f(h=h, tb=tb, o_sb=o_sb):
                    # only norm2 entries whose broadcast is already emitted
                    while len(pend2) > 1:
                        pend2.pop(0)()
                    while pend_den:
                        pend_den.pop(0)()
                    q, k, v = qkv_t[b]
                    ts = slice(tb * TB, (tb + 1) * TB)
                    o_ps = p["ps_o"].tile([128, TB], F32, name="o_ps", tag="ops")
                    eacc = p["accp"].tile([128, TB], BF16, name="eacc")
                    njs = tb * 4 + 4
                    for j in range(njs):
                        c0 = max(0, j * 128 - tb * TB)
                        cs = slice(c0, TB)
                        tcs = slice(tb * TB + c0, (tb + 1) * TB)
                        s_ps = p["ps_s"].tile([128, TB], F32, name="s_ps",
                                              tag="sps")
                        nc.tensor.matmul(
                            s_ps[:, cs],
                            k[h][:, j * 128 : (j + 1) * 128],
                            rhs=q[h][:, tcs],
                            start=True,
                            stop=True,
                        )
                        # j=0 exp writes the accumulator directly
                        e_sb = (eacc if j == 0 else
                                p["ep"].tile([128, TB], BF16, name="e_sb",
                                             tag="e"))
                        nc.scalar.activation(
                            e_sb[:, cs],
                            s_ps[:, cs],
                            mybir.ActivationFunctionType.Exp,
                            scale=SCALE,
                        )
                        if j >= tb * 4:
                            dcs = slice(c0, c0 + 128)
                            nc.vector.tensor_mul(
                                e_sb[:, dcs], e_sb[:, dcs], tri_sb[:]
                            )
                        nc.tensor.matmul(
                            o_ps[:, cs],
                            v[:, j, h * 128 : (h + 1) * 128],
                            rhs=e_sb[:, cs],
                            start=(j == 0),
                            stop=(j == njs - 1),
                        )
                        if j > 0:
                            nc.vector.tensor_add(eacc[:, cs], eacc[:, cs],
                                                 e_sb[:, cs])
                    # denominator: four N=1 matmuls put den directly in
                    # [128, 4] transposed layout (den128[p, c] = den[p*4+c])
                    # so the vector reciprocal runs wide; deferred (see
                    # pipeline comment above)
                    box = {}

                    def den(eacc=eacc, box=box):
                        d_ps = p["ps_d"].tile([128, 4], F32, name="d_ps",
                                              tag="dps")
                        et = eacc.rearrange("s (t2 g) -> s g t2", g=4)
                        for c in range(4):
                            nc.tensor.matmul(d_ps[:, c : c + 1], et[:, c],
                                             rhs=ones_sb[:],
                                             start=True, stop=True)
                        den128 = p["misc"].tile([128, 4], F32, name="den128")
                        nc.scalar.copy(den128[:], d_ps[:])
                        box["den128"] = den128

                    def norm1(box=box):
                        den128 = box["den128"]
                        nc.vector.reciprocal(den128[:], den128[:])
                        rec_row = p["misc"].tile([1, TB], F32, name="rec_row")
                        nc.sync.dma_start(out=rec_row[0:1, :], in_=den128[:])
                        rb = p["misc"].tile([128, TB], F32, name="rb")
                        nc.gpsimd.partition_broadcast(rb[:], rec_row[:])
                        box["rb"] = rb

                    def norm2(o_sb=o_sb, ts=ts, o_ps=o_ps, box=box):
                        nc.vector.tensor_mul(o_sb[:, ts], o_ps[:], box["rb"][:])
                    while pend1:
                        pend1.pop(0)()
                    pend_den.append(den)
                    pend1.append(norm1)
                    pend2.append(norm2)
                yield f
        yield flush

    wt_holder = {}

    def load_wt(ob):
        def f():
            wt = p["wp"].tile([128, KT, TB], BF16, name="wt")
            nc.sync.dma_start(out=wt[:], in_=io["w_proj_bf"][ob])
            wt_holder[ob] = wt
        return f

    def pchunk(b, ob, h):
        def f():
            wt = wt_holder[ob]
            os_ = slice(ob * TB, (ob + 1) * TB)
            y_ps = p["ps_mm"].tile([128, TB], F32, name="y_ps", tag="mmps")
            o_sb = out_sb[(b, h)]
            for kt in range(KT):
                lhsT = o_sb.rearrange("d (t2 g) -> d g t2", g=16)[:, kt]
                nc.tensor.matmul(
                    y_ps[:],
                    lhsT,
                    rhs=wt[:, kt],
                    start=(kt == 0),
                    stop=(kt == KT - 1),
                )
            y_sb = p["misc"].tile([128, TB], BF16, name="y_sb")
            nc.scalar.copy(y_sb[:], y_ps[:])
            nc.sync.dma_start(out=io["y"][b, h, :, os_], in_=y_sb[:])
        return f

    def proj_chunks_0():
        for ob in range(C // TB):
            yield load_wt(ob)
            for h in range(HL):
                yield pchunk(0, ob, h)

    def proj_chunks_1():
        # wp pool has bufs=3; after proj 0 the rotation holds ob1/ob2/ob3.
        # Visit ob1 first, reload ob0 into the freed buffer behind ob2+ob3.
        for h in range(HL):
            yield pchunk(1, 1, h)
        yield load_wt(0)
        for ob in (2, 3, 0):
            for h in range(HL):
                yield pchunk(1, ob, h)

    load_x(0, 0, first=True)()
    load_consts()
    for f in qkv_chunks(0, skip_first_load=True):
        f()
    _interleave(attn_blocks(0), qkv_chunks(1), ratio=36 / 9)
    _interleave(attn_blocks(1), proj_chunks_0(), ratio=12 / 9)
    for f in proj_chunks_1():
        f()


def _build():
    from concourse import bacc
    import concourse.mybir as mybir
    import concourse.tile as tile

    F32 = mybir.dt.float32
    BF16 = mybir.dt.bfloat16

    nc = bacc.Bacc(None, target_bir_lowering=False)
    io = {
        "x_bf": nc.dram_tensor("x_bf", [B, NTB, 128, KT, TB], BF16,
                               kind="ExternalInput"),
        "w_qkv_bf": nc.dram_tensor("w_qkv_bf", [6, 128, KT, 128], BF16,
                                   kind="ExternalInput"),
        "w_proj_bf": nc.dram_tensor("w_proj_bf", [C // TB, 128, KT, TB], BF16,
                                    kind="ExternalInput"),
        "cos2": nc.dram_tensor("cos2", [128, T], F32, kind="ExternalInput"),
        "sin2": nc.dram_tensor("sin2", [128, T], F32, kind="ExternalInput"),
        "tri": nc.dram_tensor("tri", [128, 128], BF16, kind="ExternalInput"),
        "y": nc.dram_tensor("y", [B, HL, 128, C], BF16, kind="ExternalOutput"),
    }
    with tile.TileContext(nc) as tc, ExitStack() as ctx:
        pools = {
            "const": ctx.enter_context(tc.tile_pool(name="const", bufs=1)),
            "ps_mm": ctx.enter_context(
                tc.tile_pool(name="ps_mm", bufs=2, space="PSUM")),
            "ps_s": ctx.enter_context(
                tc.tile_pool(name="ps_s", bufs=3, space="PSUM")),
            "ps_o": ctx.enter_context(
                tc.tile_pool(name="ps_o", bufs=2, space="PSUM")),
            "ps_d": ctx.enter_context(
                tc.tile_pool(name="ps_d", bufs=1, space="PSUM")),
            "xp": ctx.enter_context(tc.tile_pool(name="xp", bufs=2)),
            "qkvp": ctx.enter_context(tc.tile_pool(name="qkvp", bufs=2)),
            "ep": ctx.enter_context(tc.tile_pool(name="ep", bufs=4)),
            "accp": ctx.enter_context(tc.tile_pool(name="accp", bufs=2)),
            "outp": ctx.enter_context(tc.tile_pool(name="outp", bufs=1)),
            "wp": ctx.enter_context(tc.tile_pool(name="wp", bufs=3)),
            "misc": ctx.enter_context(tc.tile_pool(name="misc", bufs=2)),
        }
        _emit(nc, io, pools, mybir)
    nc.compile()
    return nc


def _make_executor(nc):
    import jax
    from jax.sharding import Mesh, NamedSharding, PartitionSpec
    from jax.experimental.shard_map import shard_map
    import concourse.mybir as mybir
    from concourse.bass2jax import (
        _bass_exec_p,
        install_neuronx_cc_hook,
        partition_id_tensor,
    )

    install_neuronx_cc_hook()
    partition_name = (
        nc.partition_id_tensor.name if nc.partition_id_tensor else None
    )
    in_names, out_names, out_avals, zero_outs = [], [], [], []
    for alloc in nc.m.functions[0].allocations:
        if not isinstance(alloc, mybir.MemoryLocationSet):
            continue
        name = alloc.memorylocations[0].name
        if alloc.kind == "ExternalInput":
            if name != partition_name:
                in_names.append(name)
        elif alloc.kind == "ExternalOutput":
            shape = tuple(alloc.tensor_shape)
            dtype = mybir.dt.np(alloc.dtype)
            out_names.append(name)
            out_avals.append(jax.core.ShapedArray(shape, dtype))
            zero_outs.append(np.zeros(shape, dtype))
    n_params = len(in_names)
    n_outs = len(out_avals)
    in_names.extend(out_names)
    if partition_name is not None:
        in_names.append(partition_name)
    donate = tuple(range(n_params, n_params + n_outs))

    def _body(*args):
        operands = list(args)
        if partition_name is not None:
            operands.append(partition_id_tensor())
        return tuple(
            _bass_exec_p.bind(
                *operands,
                out_avals=tuple(out_avals),
                in_names=tuple(in_names),
                out_names=tuple(out_names),
                lowering_input_output_aliases=(),
                sim_require_finite=True,
                sim_require_nnan=True,
                nc=nc,
            )
        )

    devices = jax.devices()[:N_CORES]
    assert len(devices) == N_CORES, f"need {N_CORES} cores, got {len(devices)}"
    mesh = Mesh(np.asarray(devices), ("core",))
    in_specs = (PartitionSpec("core"),) * (n_params + n_outs)
    out_specs = (PartitionSpec("core"),) * len(out_names)
    sharded = jax.jit(
        shard_map(_body, mesh=mesh, in_specs=in_specs, out_specs=out_specs,
                  check_rep=False),
        donate_argnums=donate,
        keep_unused=True,
    )

    def run(in_maps):
        per_core = [
            [np.asarray(m[name]) for name in in_names[:n_params]]
            for m in in_maps
        ]
        concat_in = [
            np.concatenate([per_core[c][i] for c in range(N_CORES)], axis=0)
            for i in range(n_params)
        ]
        concat_zeros = [
            np.zeros((N_CORES * z.shape[0], *z.shape[1:]), z.dtype)
            for z in zero_outs
        ]
        out_arrs = sharded(*concat_in, *concat_zeros)
        jax.block_until_ready(out_arrs)
        return [
            {
                name: np.asarray(out_arrs[i]).reshape(
                    N_CORES, *out_avals[i].shape
                )[c]
                for i, name in enumerate(out_names)
            }
            for c in range(N_CORES)
        ]

    return run


def _host_prep(x, w_qkv, w_proj):
    import ml_dtypes

    bf = ml_dtypes.bfloat16
    x = np.asarray(x, dtype=np.float32)
    w_qkv = np.asarray(w_qkv, dtype=np.float32)
    w_proj = np.asarray(w_proj, dtype=np.float32)

    # [B, NTB, 128(p), KT, TB]: per-partition-contiguous DMA layout;
    # x[b, tb*TB+t', kt*128+p] -> x_bf[b, tb, p, kt, t']
    x_bf = np.ascontiguousarray(
        x.reshape(B, NTB, TB, KT, 128).transpose(0, 1, 4, 3, 2)
    ).astype(bf)
    # [ob, 128(p), KT, TB]: w_proj.T[kt*128+p, ob*TB+o'] -> [ob, p, kt, o']
    w_proj_bf = np.ascontiguousarray(
        w_proj.T.reshape(KT, 128, C // TB, TB).transpose(2, 1, 0, 3)
    ).astype(bf)

    pos = np.arange(T, dtype=np.float32)[:, None]
    inv = np.exp(
        np.arange(0, D, 2, dtype=np.float32) * np.float32(-math.log(10000.0) / D)
    )
    ang = pos * inv
    sin_t = np.sin(ang).astype(np.float32).T  # [64, T]
    cos_t = np.cos(ang).astype(np.float32).T
    cos2 = np.ascontiguousarray(np.concatenate([cos_t, cos_t], axis=0))
    sin2 = np.ascontiguousarray(np.concatenate([-sin_t, sin_t], axis=0))
    tri = np.triu(np.ones((128, 128), dtype=np.float32)).astype(bf)

    in_maps = []
    for c in range(N_CORES):
        h0, h1 = 2 * c, 2 * c + 1
        blocks = []
        for base in (0, C, 2 * C):  # q, k, v feature rows
            for h in (h0, h1):
                blocks.append(w_qkv[base + h * D : base + (h + 1) * D, :])
        w_slab = np.stack(blocks, 0)  # [6, 128(d), C]
        # [6, 128(p), KT, 128(d)]: w_slab[f, d, kt*128+p] -> [f, p, kt, d]
        w_t = np.ascontiguousarray(
            w_slab.reshape(6, 128, KT, 128).transpose(0, 3, 2, 1)
        ).astype(bf)
        in_maps.append(
            {
                "x_bf": x_bf,
                "w_qkv_bf": w_t,
                "w_proj_bf": w_proj_bf,
                "cos2": cos2,
                "sin2": sin2,
                "tri": tri,
            }
        )
    return in_maps


def kernel(x, w_qkv, w_proj):
    """Full inputs in, full output out. Shards over 8 NeuronCores inside."""
    if "run" not in _CACHE:
        nc = _build()
        _CACHE["nc"] = nc
        _CACHE["run"] = _make_executor(nc)
    run = _CACHE["run"]
    in_maps = _host_prep(x, w_qkv, w_proj)
    outs = run(in_maps)
    y = np.empty((B, T, C), dtype=np.float32)
    for c in range(N_CORES):
        for hl in range(HL):
            h = 2 * c + hl
            y[:, h * 128 : (h + 1) * 128, :] = np.asarray(
                outs[c]["y"][:, hl], dtype=np.float32)
    return y


# revision 32
# speedup vs baseline: 1.0163x; 1.0163x over previous
"""TRN2 Bass kernel for nn_MultiHeadAttention_86878598464357.

reference:  qkv = x @ w_qkv.T (RoPE on q,k) -> causal softmax attention ->
            torch-faithful reshape [B,H,T,D]->[B,T,C] -> proj @ w_proj.T

Sharding (8 NeuronCores): tensor-parallel over heads, 2 heads per core.
Because the torch-faithful reshape makes output row t' depend only on head
t'//128, each core independently computes full output rows for its heads --
no collectives. Per core:
  - qkv projection for its 2 heads (bf16 matmuls, fp32 PSUM accumulation)
  - RoPE via sign-folded sin/cos tables (4 vector ops per chunk)
  - causal attention in transposed-score layout S^T[s,t] (no max-subtraction;
    scores are O(6) so exp is safe in fp32); softmax denominator via
    vector-accumulated exp tiles + one gpsimd partition_all_reduce per block
    (keeps the tensor engine free of M=1 denominator matmuls)
  - output projection with stride-16 lhsT access patterns implementing the
    reshape; w_proj slices stay resident in SBUF across both batches
Startup: x chunk 0 is DMA'd first (DGE completes transfers in FIFO order),
weights arrive in per-slice DMAs so the first matmul starts ~15us in.
Host side: transpose/cast inputs to bf16, build rope tables, scatter/gather.
"""
import math
from contextlib import ExitStack

import numpy as np

B, T, C = 2, 2048, 2048
H, D = 16, 128
HL = 2
TB = 512
NTB = T // TB
NTT = T // 128
KT = C // 128
KTH = KT // 2
SCALE = 1.0 / math.sqrt(D)
N_CORES = 8

_CACHE = {}


def _interleave(gen_a, gen_b, ratio):
    a = list(gen_a)
    bs = list(gen_b)
    bi = 0
    for i, chunk in enumerate(a):
        chunk()
        take = int(round((i + 1) * ratio)) - int(round(i * ratio))
        for _ in range(take):
            if bi < len(bs):
                bs[bi]()
                bi += 1
    while bi < len(bs):
        bs[bi]()
        bi += 1


def _emit(nc, io, p, mybir):
    """Emit the full per-core forward pass."""
    F32 = mybir.dt.float32
    BF16 = mybir.dt.bfloat16
    from concourse.bass_isa import ReduceOp

    w_sb = p["const"].tile([128, KT, 6 * 128], BF16, name="w_sb")
    cos2 = p["const"].tile([128, T], F32, name="cos2")
    sin2 = p["const"].tile([128, T], F32, name="sin2")
    tri_sb = p["const"].tile([128, 128], BF16, name="tri_sb")
    ones_sb = p["const"].tile([128, 1], BF16, name="ones_sb")
    nc.vector.memset(ones_sb[:], 1.0)



    x_holder = {}

    def load_x(b, tb, first=False):
        def f():
            xa = p["xp"].tile([128, KTH, TB], BF16, name="x_sb_a")
            xb = p["xp"].tile([128, KTH, TB], BF16, name="x_sb_b")
            src = io["x_bf"][b, tb]
            if first:
                # stage the first paired chains' inputs in dependency order:
                # quarter x, q-weight slices, rest of x
                nc.sync.dma_start(out=xa[:, 0 : KTH // 2],
                                  in_=src[:, 0 : KTH // 2])
                nc.sync.dma_start(out=w_sb[:, :, 0:128],
                                  in_=io["w_qkv_bf"][0])
                nc.sync.dma_start(out=w_sb[:, :, 128:256],
                                  in_=io["w_qkv_bf"][1])
                nc.sync.dma_start(out=xa[:, KTH // 2 : KTH],
                                  in_=src[:, KTH // 2 : KTH])
            else:
                nc.sync.dma_start(out=xa[:], in_=src[:, 0:KTH])
            nc.sync.dma_start(out=xb[:], in_=src[:, KTH:KT])
            x_holder[(b, tb)] = (xa, xb)
        return f

    def xslice(b, tb, kt):
        xa, xb = x_holder[(b, tb)]
        return (xa if kt < KTH else xb)[:, kt % KTH]

    def load_consts():
        # per-slice weight loads so early qk chains wait on little data; the
        # rope tables ship their first TB columns early (tb=0 ropes gate the
        # ps_mm recycle for the 3rd qk chain) and the rest after the weights
        nc.sync.dma_start(out=cos2[:, 0:TB], in_=io["cos2"][:, 0:TB])
        nc.sync.dma_start(out=sin2[:, 0:TB], in_=io["sin2"][:, 0:TB])
        for i in range(2, 6):
            nc.sync.dma_start(
                out=w_sb[:, :, i * 128 : (i + 1) * 128],
                in_=io["w_qkv_bf"][i],
            )
        nc.sync.dma_start(out=cos2[:, TB:T], in_=io["cos2"][:, TB:T])
        nc.sync.dma_start(out=sin2[:, TB:T], in_=io["sin2"][:, TB:T])
        nc.sync.dma_start(out=tri_sb[:], in_=io["tri"][:])

    def wslice(kt, fb):
        return w_sb[:, kt, fb * 128 : (fb + 1) * 128]

    qkv_t = {}
    out_sb = {}

    def qkv_chunks(b, skip_first_load=False):
        q = {h: p["qkvp"].tile([128, T], BF16, name=f"q_sb_{h}") for h in range(HL)}
        k = {h: p["qkvp"].tile([128, T], BF16, name=f"k_sb_{h}") for h in range(HL)}
        v = p["qkvp"].tile([128, NTT, HL * 128], BF16, name="v_sb")
        qkv_t[b] = (q, k, v)

        def rope(psum, dst, h, ts):
            # rope: dst[0:64]  = p0*cos - p1*sin
            #       dst[64:]   = p1*cos + p0*sin
            # cos2 = [cos;cos], sin2 = [-sin;sin]
            ta = p["misc"].tile([128, TB], F32, name="rope_ta")
            tb2 = p["misc"].tile([128, TB], BF16, name="rope_tb")
            nc.vector.tensor_mul(ta[:], psum[:], cos2[:, ts])
            nc.vector.tensor_mul(tb2[0:64], psum[64:128], sin2[0:64, ts])
            nc.vector.tensor_mul(tb2[64:128], psum[0:64], sin2[64:128, ts])
            nc.vector.tensor_add(dst[h][:, ts], ta[:], tb2[:])

        def qk_chunk(tb, fb, dst, h):
            def f():
                ts = slice(tb * TB, (tb + 1) * TB)
                psum = p["ps_mm"].tile([128, TB], F32, name="qk_psum", tag="mmps")
                for kt in range(KT):
                    nc.tensor.matmul(
                        psum[:],
                        wslice(kt, fb),
                        rhs=xslice(b, tb, kt),
                        start=(kt == 0),
                        stop=(kt == KT - 1),
                    )
                rope(psum, dst, h, ts)
            return f

        def qk_chunk_pair(tb, fb0_, fb1_, dst):
            # kt-interleaved pair: while the x chunk is still streaming in,
            # each arriving kt slice feeds two matmuls, matching the PE's
            # consumption rate to DMA bandwidth
            def f():
                ts = slice(tb * TB, (tb + 1) * TB)
                ps0 = p["ps_mm"].tile([128, TB], F32, name="qk_psum", tag="mmps")
                ps1 = p["ps_mm"].tile([128, TB], F32, name="qk_psum", tag="mmps")
                for kt in range(KT):
                    for ps_, fb in ((ps0, fb0_), (ps1, fb1_)):
                        nc.tensor.matmul(
                            ps_[:],
                            wslice(kt, fb),
                            rhs=xslice(b, tb, kt),
                            start=(kt == 0),
                            stop=(kt == KT - 1),
                        )
                rope(ps0, dst, 0, ts)
                rope(ps1, dst, 1, ts)
            return f

        def v_chunk(tb, tl):
            def f():
                tt = tb * 4 + tl
                psum = p["ps_mm"].tile([128, HL * 128], F32, name="v_psum",
                                       tag="mmps")
                for kt in range(KT):
                    nc.tensor.matmul(
                        psum[:],
                        xslice(b, tb, kt)[:, tl * 128 : (tl + 1) * 128],
                        rhs=w_sb[:, kt, 4 * 128 : 6 * 128],
                        start=(kt == 0),
                        stop=(kt == KT - 1),
                    )
                nc.scalar.copy(v[:, tt], psum[:])
            return f

        for tb in range(NTB):
            if not (tb == 0 and skip_first_load):
                yield load_x(b, tb)
            for fb, (dst, h) in enumerate([(q, 0), (q, 1), (k, 0), (k, 1)]):
                yield qk_chunk(tb, fb, dst, h)
            for tl in range(4):
                yield v_chunk(tb, tl)

    def attn_blocks(b):
        # the denominator chain of block n is software-pipelined across the
        # next two blocks so neither the in-order PE queue nor the vector
        # queue ever waits on it:
        #   block n+1 start: den matmul (PE, eacc ready by then) + copy
        #   block n+1 end:   reciprocal (vector) + partition broadcast
        #   block n+2 start: o_ps normalize multiply (vector)
        pend_den, pend1, pend2 = [], [], []

        def flush():
            while pend_den:
                pend_den.pop(0)()
            while pend1:
                pend1.pop(0)()
            while pend2:
                pend2.pop(0)()

        for h in range(HL):
            o_sb = p["outp"].tile([128, T], BF16, name=f"o_sb_{b}_{h}")
            out_sb[(b, h)] = o_sb
            for tb in range(NTB):
                def f(h=h, tb=tb, o_sb=o_sb):
                    # only norm2 entries whose broadcast is already emitted
                    while len(pend2) > 1:
                        pend2.pop(0)()
                    while pend_den:
                        pend_den.pop(0)()
                    q, k, v = qkv_t[b]
                    ts = slice(tb * TB, (tb + 1) * TB)
                    o_ps = p["ps_o"].tile([128, TB], F32, name="o_ps", tag="ops")
                    eacc = p["accp"].tile([128, TB], BF16, name="eacc")
                    njs = tb * 4 + 4
                    for j in range(njs):
                        c0 = max(0, j * 128 - tb * TB)
                        cs = slice(c0, TB)
                        tcs = slice(tb * TB + c0, (tb + 1) * TB)
                        s_ps = p["ps_s"].tile([128, TB], F32, name="s_ps",
                                              tag="sps")
                        nc.tensor.matmul(
                            s_ps[:, cs],
                            k[h][:, j * 128 : (j + 1) * 128],
                            rhs=q[h][:, tcs],
                            start=True,
                            stop=True,
                        )
                        # j=0 exp writes the accumulator directly
                        e_sb = (eacc if j == 0 else
                                p["ep"].tile([128, TB], BF16, name="e_sb",
                                             tag="e"))
                        nc.scalar.activation(
                            e_sb[:, cs],
                            s_ps[:, cs],
                            mybir.ActivationFunctionType.Exp,
                            scale=SCALE,
                        )
                        if j >= tb * 4:
                            dcs = slice(c0, c0 + 128)
                            nc.vector.tensor_mul(
                                e_sb[:, dcs], e_sb[:, dcs], tri_sb[:]
                            )
                        nc.tensor.matmul(
                            o_ps[:, cs],
                            v[:, j, h * 128 : (h + 1) * 128],
                            rhs=e_sb[:, cs],
                            start=(j == 0),
                            stop=(j == njs - 1),
                        )
                        if j > 0:
                            nc.vector.tensor_add(eacc[:, cs], eacc[:, cs],
                                                 e_sb[:, cs])
                    # denominator: four N=1 matmuls put den directly in
                    # [128, 4] transposed layout (den128[p, c] = den[p*4+c])
                    # so the vector reciprocal runs wide; deferred (see
                    # pipeline comment above)
                    box = {}

                    def den(eacc=eacc, box=box):
                        d_ps = p["ps_d"].tile([128, 4], F32, name="d_ps",
                                              tag="dps")
                        et = eacc.rearrange("s (t2 g) -> s g t2", g=4)
                        for c in range(4):
                            nc.tensor.matmul(d_ps[:, c : c + 1], et[:, c],
                                             rhs=ones_sb[:],
                                             start=True, stop=True)
                        den128 = p["misc"].tile([128, 4], F32, name="den128")
                        nc.scalar.copy(den128[:], d_ps[:])
                        box["den128"] = den128

                    def norm1(box=box):
                        den128 = box["den128"]
                        nc.vector.reciprocal(den128[:], den128[:])
                        rec_row = p["misc"].tile([1, TB], F32, name="rec_row")
                        nc.sync.dma_start(out=rec_row[0:1, :], in_=den128[:])
                        rb = p["misc"].tile([128, TB], F32, name="rb")
                        nc.gpsimd.partition_broadcast(rb[:], rec_row[:])
                        box["rb"] = rb

                    def norm2(o_sb=o_sb, ts=ts, o_ps=o_ps, box=box):
                        nc.vector.tensor_mul(o_sb[:, ts], o_ps[:], box["rb"][:])
                    while pend1:
                        pend1.pop(0)()
                    pend_den.append(den)
                    pend1.append(norm1)
                    pend2.append(norm2)
                yield f
        yield flush

    wt_holder = {}

    def load_wt(ob):
        def f():
            wt = p["wp"].tile([128, KT, TB], BF16, name="wt")
            nc.sync.dma_start(out=wt[:], in_=io["w_proj_bf"][ob])
            wt_holder[ob] = wt
        return f

    def pchunk(b, ob, h):
        def f():
            wt = wt_holder[ob]
            os_ = slice(ob * TB, (ob + 1) * TB)
            y_ps = p["ps_mm"].tile([128, TB], F32, name="y_ps", tag="mmps")
            o_sb = out_sb[(b, h)]
            for kt in range(KT):
                lhsT = o_sb.rearrange("d (t2 g) -> d g t2", g=16)[:, kt]
                nc.tensor.matmul(
                    y_ps[:],
                    lhsT,
                    rhs=wt[:, kt],
                    start=(kt == 0),
                    stop=(kt == KT - 1),
                )
            y_sb = p["misc"].tile([128, TB], BF16, name="y_sb")
            nc.scalar.copy(y_sb[:], y_ps[:])
            nc.sync.dma_start(out=io["y"][b, h, :, os_], in_=y_sb[:])
        return f

    def proj_chunks_0():
        for ob in range(C // TB):
            yield load_wt(ob)
            for h in range(HL):
                yield pchunk(0, ob, h)

    def proj_chunks_1():
        # wp pool has bufs=3; after proj 0 the rotation holds ob1/ob2/ob3.
        # Visit ob1 first, reload ob0 into the freed buffer behind ob2+ob3.
        for h in range(HL):
            yield pchunk(1, 1, h)
        yield load_wt(0)
        for ob in (2, 3, 0):
            for h in range(HL):
                yield pchunk(1, ob, h)

    load_x(0, 0, first=True)()
    load_consts()
    for f in qkv_chunks(0, skip_first_load=True):
        f()
    _interleave(attn_blocks(0), qkv_chunks(1), ratio=36 / 9)
    _interleave(attn_blocks(1), proj_chunks_0(), ratio=12 / 9)
    for f in proj_chunks_1():
        f()


def _build():
    from concourse import bacc
    import concourse.mybir as mybir
    import concourse.tile as tile

    F32 = mybir.dt.float32
    BF16 = mybir.dt.bfloat16

    nc = bacc.Bacc(None, target_bir_lowering=False)
    io = {
        "x_bf": nc.dram_tensor("x_bf", [B, NTB, 128, KT, TB], BF16,
                               kind="ExternalInput"),
        "w_qkv_bf": nc.dram_tensor("w_qkv_bf", [6, 128, KT, 128], BF16,
                                   kind="ExternalInput"),
        "w_proj_bf": nc.dram_tensor("w_proj_bf", [C // TB, 128, KT, TB], BF16,
                                    kind="ExternalInput"),
        "cos2": nc.dram_tensor("cos2", [128, T], F32, kind="ExternalInput"),
        "sin2": nc.dram_tensor("sin2", [128, T], F32, kind="ExternalInput"),
        "tri": nc.dram_tensor("tri", [128, 128], BF16, kind="ExternalInput"),
        "y": nc.dram_tensor("y", [B, HL, 128, C], BF16, kind="ExternalOutput"),
    }
    with tile.TileContext(nc) as tc, ExitStack() as ctx:
        pools = {
            "const": ctx.enter_context(tc.tile_pool(name="const", bufs=1)),
            "ps_mm": ctx.enter_context(
                tc.tile_pool(name="ps_mm", bufs=2, space="PSUM")),
            "ps_s": ctx.enter_context(
                tc.tile_pool(name="ps_s", bufs=3, space="PSUM")),
            "ps_o": ctx.enter_context(
                tc.tile_pool(name="ps_o", bufs=2, space="PSUM")),
            "ps_d": ctx.enter_context(
                tc.tile_pool(name="ps_d", bufs=1, space="PSUM")),
            "xp": ctx.enter_context(tc.tile_pool(name="xp", bufs=2)),
            "qkvp": ctx.enter_context(tc.tile_pool(name="qkvp", bufs=2)),
            "ep": ctx.enter_context(tc.tile_pool(name="ep", bufs=4)),
            "accp": ctx.enter_context(tc.tile_pool(name="accp", bufs=2)),
            "outp": ctx.enter_context(tc.tile_pool(name="outp", bufs=1)),
            "wp": ctx.enter_context(tc.tile_pool(name="wp", bufs=3)),
            "misc": ctx.enter_context(tc.tile_pool(name="misc", bufs=2)),
        }
        _emit(nc, io, pools, mybir)
    nc.compile()
    return nc


def _make_executor(nc):
    import jax
    from jax.sharding import Mesh, NamedSharding, PartitionSpec
    from jax.experimental.shard_map import shard_map
    import concourse.mybir as mybir
    from concourse.bass2jax import (
        _bass_exec_p,
        install_neuronx_cc_hook,
        partition_id_tensor,
    )

    install_neuronx_cc_hook()
    partition_name = (
        nc.partition_id_tensor.name if nc.partition_id_tensor else None
    )
    in_names, out_names, out_avals, zero_outs = [], [], [], []
    for alloc in nc.m.functions[0].allocations:
        if not isinstance(alloc, mybir.MemoryLocationSet):
            continue
        name = alloc.memorylocations[0].name
        if alloc.kind == "ExternalInput":
            if name != partition_name:
                in_names.append(name)
        elif alloc.kind == "ExternalOutput":
            shape = tuple(alloc.tensor_shape)
            dtype = mybir.dt.np(alloc.dtype)
            out_names.append(name)
            out_avals.append(jax.core.ShapedArray(shape, dtype))
            zero_outs.append(np.zeros(shape, dtype))
    n_params = len(in_names)
    n_outs = len(out_avals)
    in_names.extend(out_names)
    if partition_name is not None:
        in_names.append(partition_name)
    donate = tuple(range(n_params, n_params + n_outs))

    def _body(*args):
        operands = list(args)
        if partition_name is not None:
            operands.append(partition_id_tensor())
        return tuple(
            _bass_exec_p.bind(
                *operands,
                out_avals=tuple(out_avals),
                in_names=tuple(in_names),
                out_names=tuple(out_names),
                lowering_input_output_aliases=(),
                sim_require_finite=True,
                sim_require_nnan=True,
                nc=nc,
            )
        )

    devices = jax.devices()[:N_CORES]
    assert len(devices) == N_CORES, f"need {N_CORES} cores, got {len(devices)}"
    mesh = Mesh(np.asarray(devices), ("core",))
    in_specs = (PartitionSpec("core"),) * (n_params + n_outs)
    out_specs = (PartitionSpec("core"),) * len(out_names)
    sharded = jax.jit(
        shard_map(_body, mesh=mesh, in_specs=in_specs, out_specs=out_specs,
                  check_rep=False),
        donate_argnums=donate,
        keep_unused=True,
    )

    def run(in_maps):
        per_core = [
            [np.asarray(m[name]) for name in in_names[:n_params]]
            for m in in_maps
        ]
        concat_in = [
            np.concatenate([per_core[c][i] for c in range(N_CORES)], axis=0)
            for i in range(n_params)
        ]
        concat_zeros = [
            np.zeros((N_CORES * z.shape[0], *z.shape[1:]), z.dtype)
            for z in zero_outs
        ]
        out_arrs = sharded(*concat_in, *concat_zeros)
        jax.block_until_ready(out_arrs)
        return [
            {
                name: np.asarray(out_arrs[i]).reshape(
                    N_CORES, *out_avals[i].shape
                )[c]
                for i, name in enumerate(out_names)
            }
            for c in range(N_CORES)
        ]

    return run


def _host_prep(x, w_qkv, w_proj):
    import ml_dtypes

    bf = ml_dtypes.bfloat16
    x = np.asarray(x, dtype=np.float32)
    w_qkv = np.asarray(w_qkv, dtype=np.float32)
    w_proj = np.asarray(w_proj, dtype=np.float32)

    # [B, NTB, 128(p), KT, TB]: per-partition-contiguous DMA layout;
    # x[b, tb*TB+t', kt*128+p] -> x_bf[b, tb, p, kt, t']
    x_bf = np.ascontiguousarray(
        x.reshape(B, NTB, TB, KT, 128).transpose(0, 1, 4, 3, 2)
    ).astype(bf)
    # [ob, 128(p), KT, TB]: w_proj.T[kt*128+p, ob*TB+o'] -> [ob, p, kt, o']
    w_proj_bf = np.ascontiguousarray(
        w_proj.T.reshape(KT, 128, C // TB, TB).transpose(2, 1, 0, 3)
    ).astype(bf)

    pos = np.arange(T, dtype=np.float32)[:, None]
    inv = np.exp(
        np.arange(0, D, 2, dtype=np.float32) * np.float32(-math.log(10000.0) / D)
    )
    ang = pos * inv
    sin_t = np.sin(ang).astype(np.float32).T  # [64, T]
    cos_t = np.cos(ang).astype(np.float32).T
    cos2 = np.ascontiguousarray(np.concatenate([cos_t, cos_t], axis=0))
    sin2 = np.ascontiguousarray(np.concatenate([-sin_t, sin_t], axis=0))
    tri = np.triu(np.ones((128, 128), dtype=np.float32)).astype(bf)

    in_maps = []
    for c in range(N_CORES):
        h0, h1 = 2 * c, 2 * c + 1
        blocks = []
        for base in (0, C, 2 * C):  # q, k, v feature rows
            for h in (h0, h1):
                blocks.append(w_qkv[base + h * D : base + (h + 1) * D, :])
        w_slab = np.stack(blocks, 0)  # [6, 128(d), C]
        # [6, 128(p), KT, 128(d)]: w_slab[f, d, kt*128+p] -> [f, p, kt, d]
        w_t = np.ascontiguousarray(
            w_slab.reshape(6, 128, KT, 128).transpose(0, 3, 2, 1)
        ).astype(bf)
        in_maps.append(
            {
                "x_bf": x_bf,
                "w_qkv_bf": w_t,
                "w_proj_bf": w_proj_bf,
                "cos2": cos2,
                "sin2": sin2,
                "tri": tri,
            }
        )
    return in_maps


def kernel(x, w_qkv, w_proj):
    """Full inputs in, full output out. Shards over 8 NeuronCores inside."""
    if "run" not in _CACHE:
        nc = _build()
        _CACHE["nc"] = nc
        _CACHE["run"] = _make_executor(nc)
    run = _CACHE["run"]
    in_maps = _host_prep(x, w_qkv, w_proj)
    outs = run(in_maps)
    y = np.empty((B, T, C), dtype=np.float32)
    for c in range(N_CORES):
        for hl in range(HL):
            h = 2 * c + hl
            y[:, h * 128 : (h + 1) * 128, :] = np.asarray(
                outs[c]["y"][:, hl], dtype=np.float32)
    return y
